# revision 1
# baseline (speedup 1.0000x reference)
"""Trainium2 Bass kernel for LowLightAdaptiveCNNBlock (moe_routing).

Full inputs in, full outputs out. Data-parallel over batch on 8 NeuronCores
(2 samples/core), params replicated.

Per-core program (SPMD):
  - routing path via pooled-sum algebra: mean(BN(dwconv(x,w)+b)) is linear in 9
    per-channel window sums, each expressible from the total sum, border
    row/col sums and corners -> a handful of tiny reduces instead of a conv.
  - softmax routing weights -> expert kernel/bias mixing via tiny PE matmuls
    (contraction over E=8).
  - main per-sample depthwise 3x3 conv: per (sample, channel-block) tile,
    9 taps as diagonal fp32r matmuls accumulating in PSUM plus an identity
    matmul adding the residual x; BN scale folded into the tap weights; the
    epilogue is one ScalarE relu(psum + off) PSUM->SBUF, then DMA out.

x layout in SBUF: per tile [128ch, 9412], rows of 98 (1 zero pad + 96 data +
1 zero pad); element (r, w) at offset 1 + 98*r + w.
"""
import sys, os

sys.path.insert(0, "/opt/trn_rl_repo")

KPART = os.environ.get("KPART", "full")

import numpy as np
import concourse.bass as bass
import concourse.bacc as bacc
import concourse.tile as tile
from concourse import mybir
from concourse.bass_utils import run_bass_kernel_spmd

f32 = mybir.dt.float32
f32r = mybir.dt.float32r

EPS = 1e-5
B, C, H, W, E = 16, 256, 96, 96, 8
NCORES = 8
BLOC = B // NCORES          # samples per core
NCB = C // 128              # channel blocks
WP = W + 2                  # padded row stride (98)
XTD = WP * H                # 9408 = padded data span
XTF = XTD + 4               # tile free size (slack for tap slices)
ROWCH = 5                   # rows per conv chunk
HW = H * W

TAPS = [(sh, sw) for sh in (-1, 0, 1) for sw in (-1, 0, 1)]

CHUNKS = []
_r = 0
while _r < H:
    _nr = min(ROWCH, H - _r)
    CHUNKS.append((_r, _nr))
    _r += _nr


def _build_program():
    nc = bacc.Bacc("TRN2", target_bir_lowering=False, debug=False,
                   num_devices=NCORES)

    x_d = nc.dram_tensor("x", [BLOC, C, H, WP], f32r,
                         kind="ExternalInput").ap()
    pp_d = nc.dram_tensor("pp", [NCB, 128, 19], f32, kind="ExternalInput").ap()
    emb_d = nc.dram_tensor("emb", [8, NCB * 10 * 128], f32,
                           kind="ExternalInput").ap()
    db_d = nc.dram_tensor("db", [1, E], f32, kind="ExternalInput").ap()
    id_d = nc.dram_tensor("ident", [128, 128], f32r, kind="ExternalInput").ap()
    out_d = nc.dram_tensor("out", [BLOC, C, H, W], f32,
                           kind="ExternalOutput").ap()

    Relu = mybir.ActivationFunctionType.Relu
    Exp = mybir.ActivationFunctionType.Exp
    add_op = mybir.AluOpType.add
    mult_op = mybir.AluOpType.mult
    AX = mybir.AxisListType.X
    AXY = mybir.AxisListType.XY

    with tile.TileContext(nc) as tc:
        with tc.tile_pool(name="const", bufs=1) as cpool, \
             tc.tile_pool(name="xp", bufs=3) as xpool, \
             tc.tile_pool(name="small", bufs=4) as spool, \
             tc.tile_pool(name="rp", bufs=1) as rpool, \
             tc.tile_pool(name="dg", bufs=2) as dgpool, \
             tc.tile_pool(name="st", bufs=6) as stpool, \
             tc.tile_pool(name="pc", bufs=6, space="PSUM") as pconv, \
             tc.tile_pool(name="pxs", bufs=2, space="PSUM") as psmall:

            # ---- constants
            pp_t = [cpool.tile([128, 19], f32, tag=f"pp{cb}", name=f"pp_t{cb}")
                    for cb in range(NCB)]
            for cb in range(NCB):
                nc.sync.dma_start(pp_t[cb][:], pp_d[cb])
            emb_t = cpool.tile([8, NCB * 10 * 128], f32, tag="emb")
            nc.sync.dma_start(emb_t[:], emb_d)
            db_t = cpool.tile([1, E], f32, tag="db")
            nc.sync.dma_start(db_t[:], db_d)
            id_t = cpool.tile([128, 128], f32r, tag="ident")
            nc.sync.dma_start(id_t[:], id_d)
            ones_t = cpool.tile([1, 1], f32, tag="ones")
            nc.vector.memset(ones_t[:], 1.0)

            # persistent routing activations r[b][cb] : [128, 1]
            r_t = [[rpool.tile([128, 1], f32, tag=f"r{b}{cb}", name=f"r_t{b}{cb}")
                    for cb in range(NCB)] for b in range(BLOC)]

            xt = [[None] * NCB for _ in range(BLOC)]

            def load_and_route(b, cb):
                t = xpool.tile([128, XTF], f32r, tag="xt")
                xt[b][cb] = t
                # x arrives host-padded: rows of 98 with zero cols 0 and 97
                nc.sync.dma_start(t[:, 0:XTD],
                                  x_d[b, cb * 128:(cb + 1) * 128])

                tf = t[:].bitcast(f32)
                colv = tf[:, 0:XTD].rearrange("p (r c) -> p r c", c=WP)
                # routing base sums U: [T, R0, R95, C0, C95, x00, x0_95,
                #                       x95_0, x95_95]
                U = spool.tile([128, 16], f32, tag="U")
                nc.vector.tensor_reduce(U[:, 0:1], colv, axis=AXY, op=add_op)
                nc.vector.tensor_reduce(U[:, 1:2], colv[:, 0:1, 1:97],
                                        axis=AX, op=add_op)
                nc.vector.tensor_reduce(U[:, 2:3], colv[:, 95:96, 1:97],
                                        axis=AX, op=add_op)
                nc.vector.tensor_reduce(U[:, 3:4], colv[:, :, 1:2],
                                        axis=AXY, op=add_op)
                nc.vector.tensor_reduce(U[:, 4:5], colv[:, :, 96:97],
                                        axis=AXY, op=add_op)
                nc.vector.tensor_copy(U[:, 5:7], tf[:, 1:97:95])
                nc.vector.tensor_copy(U[:, 7:9],
                                      tf[:, 1 + WP * 95:97 + WP * 95:95])

                scr9 = spool.tile([128, 9], f32, tag="scr9")
                m_t = spool.tile([128, 1], f32, tag="m")
                nc.vector.tensor_tensor(scr9[:], U[:, 0:9], pp_t[cb][:, 0:9],
                                        op=mult_op)
                nc.vector.tensor_reduce(m_t[:], scr9[:], axis=AX, op=add_op)
                nc.scalar.activation(r_t[b][cb][:], m_t[:], Relu,
                                     bias=pp_t[cb][:, 9:10], scale=1.0)

            def routing_tail(b):
                """softmax over experts; returns transposed weights wT [8,1]."""
                pl = psmall.tile([1, E], f32, tag="psm")
                for cb in range(NCB):
                    nc.tensor.matmul(pl[:], r_t[b][cb][:], pp_t[cb][:, 11:19],
                                     start=(cb == 0), stop=(cb == NCB - 1))
                lg = spool.tile([1, E], f32, tag="lg")
                nc.vector.tensor_tensor(lg[:], pl[:], db_t[:], op=add_op)
                mx = spool.tile([1, 1], f32, tag="mx")
                nc.vector.reduce_max(mx[:], lg[:], axis=AX)
                mxn = spool.tile([1, 1], f32, tag="mxn")
                nc.vector.tensor_scalar_mul(mxn[:], mx[:], -1.0)
                ex = spool.tile([1, E], f32, tag="ex")
                nc.scalar.activation(ex[:], lg[:], Exp, bias=mxn[:], scale=1.0)
                sm = spool.tile([1, 1], f32, tag="sm")
                nc.vector.reduce_sum(sm[:], ex[:], axis=AX)
                rs = spool.tile([1, 1], f32, tag="rs")
                nc.vector.reciprocal(rs[:], sm[:])
                wsm = spool.tile([1, E], f32, tag="wsm")
                nc.vector.tensor_scalar_mul(wsm[:], ex[:], rs[:])
                pw = psmall.tile([E, 1], f32, tag="psm")
                nc.tensor.matmul(pw[:], wsm[:], ones_t[:], start=True,
                                 stop=True)
                wT = spool.tile([E, 1], f32, tag="wT")
                nc.scalar.copy(wT[:], pw[:])
                return wT

            def mix_kb(b, cb, wT):
                """mixed+scaled tap weights -> diag [128, 9*128] f32r and
                off [128, 1]."""
                pk = psmall.tile([128, 10], f32, tag="psm")
                for t in range(10):
                    sl = emb_t[:, (cb * 10 + t) * 128:(cb * 10 + t + 1) * 128]
                    nc.tensor.matmul(pk[:, t:t + 1], sl, wT[:],
                                     start=True, stop=True)
                kb = spool.tile([128, 10], f32, tag="kb")
                nc.scalar.copy(kb[:], pk[:])
                off = spool.tile([128, 1], f32, tag="off")
                nc.vector.tensor_tensor(off[:], kb[:, 9:10],
                                        pp_t[cb][:, 10:11], op=add_op)
                diag = dgpool.tile([128, 9 * 128], f32r, tag="diag")
                idf = id_t[:].bitcast(f32)
                for t in range(9):
                    nc.vector.tensor_scalar_mul(
                        diag[:, t * 128:(t + 1) * 128], idf, kb[:, t:t + 1])
                return diag, off

            def conv_tile(b, cb, diag, off):
                t = xt[b][cb]
                for (r0, nr) in CHUNKS:
                    n = nr * W
                    ps = pconv.tile([128, ROWCH * W], f32, tag="pc")
                    base = WP * r0
                    iv = t[:, base + 1:base + 1 + WP * nr].rearrange(
                        "p (r c) -> p r c", c=WP)[:, :, 0:96]
                    mms = [(id_t[:], iv, ps[:, 0:n])]
                    for t_i, (sh, sw) in enumerate(TAPS):
                        rr0 = max(r0, -sh)
                        rr1 = min(r0 + nr, H - sh)
                        nrr = rr1 - rr0
                        if nrr <= 0:
                            continue
                        bb = WP * (rr0 + sh) + sw
                        ivt = t[:, bb + 1:bb + 1 + WP * nrr].rearrange(
                            "p (r c) -> p r c", c=WP)[:, :, 0:96]
                        ov = ps[:, (rr0 - r0) * W:(rr0 - r0) * W + nrr * W]
                        mms.append((diag[:, t_i * 128:(t_i + 1) * 128],
                                    ivt, ov))
                    for i, (lhsT, rhs, ov) in enumerate(mms):
                        nc.tensor.matmul(ov, lhsT, rhs, start=(i == 0),
                                         stop=(i == len(mms) - 1))
                    stage = stpool.tile([128, ROWCH * W], f32, tag="st")
                    nc.scalar.activation(stage[:, 0:n], ps[:, 0:n], Relu,
                                         bias=off[:], scale=1.0)
                    nc.sync.dma_start(
                        out_d[b, cb * 128:(cb + 1) * 128, r0:r0 + nr],
                        stage[:, 0:n])

            # ---- main schedule
            for b in range(BLOC):
                for cb in range(NCB):
                    load_and_route(b, cb)
                if KPART == "route":
                    continue
                wT = routing_tail(b)
                for cb in range(NCB):
                    diag, off = mix_kb(b, cb, wT)
                    if KPART == "mix":
                        continue
                    conv_tile(b, cb, diag, off)

    if not nc.is_finalized():
        nc.finalize()
    return nc


_NC = None


def _get_nc():
    global _NC
    if _NC is None:
        _NC = _build_program()
    return _NC


def _host_prep(kernel_embed, bias_embed, cls_conv_w, cls_conv_b,
               cls_bn_gamma, cls_bn_beta, cls_bn_mean, cls_bn_var,
               cls_dense_w, cls_dense_b, bn_gamma, bn_beta, bn_mean, bn_var):
    inv = bn_gamma / np.sqrt(bn_var + EPS)
    shift = bn_beta - bn_mean * inv
    cls_inv = cls_bn_gamma / np.sqrt(cls_bn_var + EPS)
    cls_shift = cls_bn_beta - cls_bn_mean * cls_inv

    cls_w9 = cls_conv_w.reshape(C, 9)
    A = (cls_inv[:, None] * cls_w9) / HW                      # (C, 9)
    d = cls_inv * cls_conv_b + cls_shift                      # (C,)

    # window-sum decomposition: S_ij = T - rho - gamma + kappa
    # base order: [T, R0, R95, C0, C95, x00, x0_95, x95_0, x95_95]
    C9 = np.zeros((C, 9), np.float64)
    for i, sh in enumerate((-1, 0, 1)):
        for j, sw in enumerate((-1, 0, 1)):
            a = A[:, i * 3 + j].astype(np.float64)
            C9[:, 0] += a
            if sh == -1:
                C9[:, 2] -= a
            elif sh == 1:
                C9[:, 1] -= a
            if sw == -1:
                C9[:, 4] -= a
            elif sw == 1:
                C9[:, 3] -= a
            if sh != 0 and sw != 0:
                rr = 95 if sh == -1 else 0
                cc = 95 if sw == -1 else 0
                idx = 5 + (2 if rr == 95 else 0) + (1 if cc == 95 else 0)
                C9[:, idx] += a
    C9 = C9.astype(np.float32)

    pp = np.zeros((NCB, 128, 19), np.float32)
    for cb in range(NCB):
        s = slice(cb * 128, (cb + 1) * 128)
        pp[cb, :, 0:9] = C9[s]
        pp[cb, :, 9] = d[s]
        pp[cb, :, 10] = shift[s]
        pp[cb, :, 11:19] = cls_dense_w[:, s].T

    ke = kernel_embed.reshape(E, C, 9) * inv[None, :, None]
    be = bias_embed * inv[None, :]
    emb = np.zeros((E, NCB * 10 * 128), np.float32)
    for cb in range(NCB):
        s = slice(cb * 128, (cb + 1) * 128)
        for t in range(9):
            emb[:, (cb * 10 + t) * 128:(cb * 10 + t + 1) * 128] = ke[:, s, t]
        emb[:, (cb * 10 + 9) * 128:(cb * 10 + 10) * 128] = be[:, s]

    db = cls_dense_b.reshape(1, E).astype(np.float32)
    ident = np.eye(128, dtype=np.float32)
    return pp, emb, db, ident


def kernel(x, kernel_embed, bias_embed, cls_conv_w, cls_conv_b,
           cls_bn_gamma, cls_bn_beta, cls_bn_mean, cls_bn_var,
           cls_dense_w, cls_dense_b, bn_gamma, bn_beta, bn_mean, bn_var,
           _trace=False, _trace_kwargs=None):
    x = np.asarray(x, dtype=np.float32)
    xp_full = np.zeros((B, C, H, WP), np.float32)
    xp_full[:, :, :, 1:97] = x
    args = [np.asarray(a, dtype=np.float32) for a in
            (kernel_embed, bias_embed, cls_conv_w, cls_conv_b,
             cls_bn_gamma, cls_bn_beta, cls_bn_mean, cls_bn_var,
             cls_dense_w, cls_dense_b, bn_gamma, bn_beta, bn_mean, bn_var)]
    pp, emb, db, ident = _host_prep(*args)

    nc = _get_nc()
    in_maps = []
    for core in range(NCORES):
        xs = np.ascontiguousarray(xp_full[core * BLOC:(core + 1) * BLOC])
        in_maps.append({"x": xs, "pp": pp, "emb": emb, "db": db,
                        "ident": ident})
    kw = {}
    if _trace:
        kw["trace"] = True
        if _trace_kwargs:
            kw.update(_trace_kwargs)
    res = run_bass_kernel_spmd(nc, in_maps, core_ids=list(range(NCORES)), **kw)
    out = np.concatenate([res.results[i]["out"] for i in range(NCORES)],
                         axis=0)
    if _trace:
        return out, res
    return out

